# revision 23
# baseline (speedup 1.0000x reference)
"""KAN layer (piecewise-linear spline lookup) on 8 TRN2 NeuronCores.

Math: per (batch b, feature f), with u = (x+3)/h, h = 6/63 (64 uniform knots):
    y[b,o] = sum_f C[f,0,o] + sum_{k=0}^{62} s_k[f,o] * clip(u - k, 0, 1)
with s_k = C[:,k+1,:] - C[:,k,:] and un-clamped edges (k=0 below, k=62 above)
for the linear extrapolation the reference's index-clipping implies.

The clip basis is numerically robust in fp16: saturated entries are exact
integers, only the single fractional knot per (b,f) rounds. Each basis tile
is produced in ONE fused DVE op via the centered clamp identity
    clip(u-k, 0, 1) = clamp(u - 8c - 4, j-4, j-3) + (j-4),   k = 8c+j,
i.e. tensor_scalar(max j-4, min j-3) on a per-chunk pre-shifted u tile
(ScalarE produces the 8 shifted fp16 u-chunks straight from x). The constant
offsets sum_k (j(k)-4) s_k fold into the base term, computed once on device:
    base = sum_g C[g] - 4 C0 - 8 (C8+C16+...+C56) - 3 C63.

All 63 clamp tiles are produced on DVE as 8 fused full-width tensor_scalar
ops (fp16 in/out -> 4x perf mode); GPSIMD/Pool is deliberately unused (its
per-op ucode launch overhead measures ~7us on HW). The base term is folded
into the bias vector by a one-time setup matmul, so the contraction is 63
accumulating fp16 TensorE matmuls (K=128 features on partitions, M=64
outputs, N=512 batch): 32 into PSUM partitions 0:64 (knots 0..31) and 31
into 64:128 (knots 32..62) — two PE column groups that execute concurrently
on hardware. ScalarE + DVE then merge the PSUM halves and add the bias.

Steady-state structure: the coeff table DMA + slope/base-table prep run once
(setup); each rep streams x in, computes, streams y out — weights stay
resident in SBUF, as in real inference.

Sharding: data-parallel — batch 4096 split into 8 shards of 512; coeff and
bias replicated to every core (per the sharding hint). Host does layout-only
prep (transpose/reshape/slice).
"""

import numpy as np

import concourse.bass as bass
import concourse.mybir as mybir
import concourse.tile as tile
from concourse import bacc
from concourse.bass_utils import run_bass_kernel_spmd

F32 = mybir.dt.float32
F16 = mybir.dt.float16
ALU = mybir.AluOpType
ACTF = mybir.ActivationFunctionType

IN_DIM = 128     # features (partition dim)
OUT_DIM = 64
GRID = 64
B = 4096
N_CORES = 8
BS = B // N_CORES          # 512 batch rows per core
X_MIN, X_MAX = -3.0, 3.0
H = (X_MAX - X_MIN) / (GRID - 1)          # 6/63
INV_H = 1.0 / H
U_OFF = -X_MIN / H                         # +3/h = 31.5
NK = GRID - 1                              # 63 slope knots, k = 0..62
NC_CH = 8                                  # chunks of 8 knots
CW = OUT_DIM                               # 64 floats per grid row

# (c, j) -> knot k = 8c + j; valid iff k <= 62 (i.e. not (7,7)).


def _valid(c, j):
    return 8 * c + j <= 62


def build_program(reps: int = 1, variant: str = "full"):
    nc = bacc.Bacc(
        "TRN2",
        target_bir_lowering=False,
        debug=False,
        num_devices=N_CORES,
    )
    xT_d = nc.dram_tensor("xT", [IN_DIM, BS], F32, kind="ExternalInput")
    coeff_d = nc.dram_tensor("coeff", [IN_DIM, GRID * CW], F32, kind="ExternalInput")
    bias_d = nc.dram_tensor("bias", [OUT_DIM, 1], F32, kind="ExternalInput")
    yT_d = nc.dram_tensor("yT", [OUT_DIM, BS], F32, kind="ExternalOutput")

    with tile.TileContext(nc) as tc:
        _emit(tc, yT_d.ap(), xT_d.ap(), coeff_d.ap(), bias_d.ap(), reps, variant)

    nc.compile()
    return nc


def _emit(tc, yT, xT, coeffR, biasd, reps, variant="full"):
    nc = tc.nc
    dt16 = mybir.dt.bfloat16 if variant == "bf16" else F16

    with tc.tile_pool(name="persist", bufs=1) as ppool:
        W16 = ppool.tile([IN_DIM, NK * CW], dt16, tag="W16")      # slopes fp16
        bt = ppool.tile([OUT_DIM, 1], F32, tag="bt")
        b62 = ppool.tile([IN_DIM, 1], F32, tag="b62")             # U_OFF - 62

        # ---------------- one-time setup: tables from coeff ----------------
        with (
            tc.tile_pool(name="setup", bufs=1) as spool,
            tc.tile_pool(name="setup_ps", bufs=1, space="PSUM") as sps,
        ):
            C = spool.tile([IN_DIM, GRID * CW], F32, tag="C")
            n_dma = 8
            cw = GRID * CW // n_dma
            for d in range(n_dma):
                nc.sync.dma_start(
                    out=C[:, d * cw : (d + 1) * cw],
                    in_=coeffR[:, d * cw : (d + 1) * cw],
                )
            nc.sync.dma_start(out=bt[:], in_=biasd[:, :])
            nc.vector.memset(b62[:], U_OFF - 62.0)

            # slopes: s_k = C[k+1] - C[k], cast to fp16
            nc.vector.tensor_tensor(
                out=W16[:], in0=C[:, CW:], in1=C[:, : NK * CW], op=ALU.subtract
            )

            # S = sum_g C[g]  (pairwise tree over the 64 grid rows)
            t1 = spool.tile([IN_DIM, 32 * CW], F32, tag="t1")
            nc.vector.tensor_tensor(
                out=t1[:], in0=C[:, : 32 * CW], in1=C[:, 32 * CW :], op=ALU.add
            )
            t2 = spool.tile([IN_DIM, 16 * CW], F32, tag="t2")
            nc.vector.tensor_tensor(
                out=t2[:], in0=t1[:, : 16 * CW], in1=t1[:, 16 * CW :], op=ALU.add
            )
            t3 = spool.tile([IN_DIM, 8 * CW], F32, tag="t3")
            nc.vector.tensor_tensor(
                out=t3[:], in0=t2[:, : 8 * CW], in1=t2[:, 8 * CW :], op=ALU.add
            )
            t4 = spool.tile([IN_DIM, 4 * CW], F32, tag="t4")
            nc.vector.tensor_tensor(
                out=t4[:], in0=t3[:, : 4 * CW], in1=t3[:, 4 * CW :], op=ALU.add
            )
            t5 = spool.tile([IN_DIM, 2 * CW], F32, tag="t5")
            nc.vector.tensor_tensor(
                out=t5[:], in0=t4[:, : 2 * CW], in1=t4[:, 2 * CW :], op=ALU.add
            )
            S = spool.tile([IN_DIM, CW], F32, tag="S")
            nc.vector.tensor_tensor(
                out=S[:], in0=t5[:, :CW], in1=t5[:, CW:], op=ALU.add
            )

            # T8 = C8 + C16 + ... + C56 (7 rows)
            T8 = spool.tile([IN_DIM, CW], F32, tag="T8")
            nc.vector.tensor_tensor(
                out=T8[:],
                in0=C[:, 8 * CW : 9 * CW],
                in1=C[:, 16 * CW : 17 * CW],
                op=ALU.add,
            )
            for m in (24, 32, 40, 48, 56):
                nc.vector.tensor_tensor(
                    out=T8[:], in0=T8[:], in1=C[:, m * CW : (m + 1) * CW], op=ALU.add
                )

            # base = S - 4*C0 - 8*T8 - 3*C63  (then cast fp16 via ACT copy,
            # which also makes the ACT table resident before the rep loop)
            bfp = spool.tile([IN_DIM, CW], F32, tag="bfp")
            nc.vector.scalar_tensor_tensor(
                out=bfp[:], in0=C[:, :CW], scalar=-4.0, in1=S[:],
                op0=ALU.mult, op1=ALU.add,
            )
            nc.vector.scalar_tensor_tensor(
                out=bfp[:], in0=T8[:], scalar=-8.0, in1=bfp[:],
                op0=ALU.mult, op1=ALU.add,
            )
            nc.vector.scalar_tensor_tensor(
                out=bfp[:], in0=C[:, 63 * CW :], scalar=-1.0, in1=bfp[:],
                op0=ALU.mult, op1=ALU.add,
            )
            nc.vector.scalar_tensor_tensor(
                out=bfp[:], in0=C[:, 62 * CW : 63 * CW], scalar=-2.0, in1=bfp[:],
                op0=ALU.mult, op1=ALU.add,
            )
            # fold base into the bias: bt += sum_f base[f, :]
            ones_col = spool.tile([IN_DIM, 1], F32, tag="ones_col")
            nc.vector.memset(ones_col[:], 1.0)
            psb = sps.tile([OUT_DIM, 1], F32, tag="psb")
            nc.tensor.matmul(psb[:], bfp[:], ones_col[:], start=True, stop=True)
            nc.vector.tensor_tensor(out=bt[:], in0=bt[:], in1=psb[:], op=ALU.add)
            # make the ACT table resident before the rep loop
            nc.scalar.activation(bfp[:], bfp[:], ACTF.Copy)

        # ---------------- steady-state rep loop ----------------
        with (
            tc.tile_pool(name="xp", bufs=3) as xpool,
            tc.tile_pool(name="up", bufs=3) as upool,
            tc.tile_pool(name="bp", bufs=2) as bpool,
            tc.tile_pool(name="yp", bufs=3) as ypool,
            tc.tile_pool(name="ps", bufs=3, space="PSUM") as pspool,
        ):
            for _ in range(reps):
                _emit_rep(nc, xpool, upool, bpool, ypool, pspool,
                          W16, bt, b62, xT, yT, dt16, variant)


def _emit_rep(nc, xpool, upool, bpool, ypool, pspool,
              W16, bt, b62, xT, yT, dt16=F16, variant="full"):
    do_ew = variant in ("full", "bf16", "nomm")
    do_mm = variant in ("full", "bf16", "noew")
    xt = xpool.tile([IN_DIM, BS], F32, tag="xt")
    nc.sync.dma_start(out=xt[:], in_=xT[:, :])

    # u chunks: U8[:, c*BS:(c+1)*BS] = (x * 1/h) + (31.5 - 8c - 4), fp16
    U8 = upool.tile([IN_DIM, NC_CH * BS], dt16, tag="U8")
    for c in range(NC_CH) if (do_ew or variant == "noew") else ():
        nc.scalar.activation(
            U8[:, c * BS : (c + 1) * BS], xt[:], ACTF.Copy,
            bias=U_OFF - 8.0 * c - 4.0, scale=INV_H,
        )
    # basis tiles: B_j[:, c*BS:(c+1)*BS] = clamp(u_c, j-4, j-3)   (fp16)
    Bj = []
    for j in range(8):
        bj = bpool.tile([IN_DIM, NC_CH * BS], dt16, tag=f"B{j}")
        Bj.append(bj)

    if do_ew:
        # knot 62 (uncapped top knot): relu(u-62) on ScalarE, offset-free
        nc.scalar.activation(
            Bj[6][:, 7 * BS : 8 * BS], xt[:], ACTF.Relu,
            bias=b62[:, :], scale=INV_H,
        )

    def clamp_single(eng, j, c):
        k = 8 * c + j
        jc = float(j - 4)
        src = U8[:, c * BS : (c + 1) * BS]
        dst = Bj[j][:, c * BS : (c + 1) * BS]
        if k == 0:
            eng.tensor_scalar(dst, src, jc + 1.0, None, ALU.min)
        elif k == 62:
            eng.tensor_scalar(dst, src, jc, None, ALU.max)
        else:
            eng.tensor_scalar(dst, src, jc, jc + 1.0, ALU.max, ALU.min)

    # All clamps on DVE (Pool/GPSIMD has ~7us ucode-launch overhead per op —
    # measured, unusable here). One fused clamp per j over every valid chunk;
    # the edge knots 0 (no lower clamp) and 62 (no cap) are split out.
    if do_ew:
        clamp_single(nc.vector, 0, 0)            # knot 0: min only
    for j in range(8) if do_ew else ():
        jc = float(j - 4)
        lo = 1 if j == 0 else 0                  # knot 0 handled above
        hi = 7 if j < 6 else 6                   # knot 62 special; (7,7) absent
        nc.vector.tensor_scalar(
            Bj[j][:, lo * BS : (hi + 1) * BS],
            U8[:, lo * BS : (hi + 1) * BS],
            jc, jc + 1.0, ALU.max, ALU.min,
        )

    # ---- matmuls: psum[0:64] <- knots c0..3 ; psum[64:128] <- base + c4..7
    ps = pspool.tile([2 * OUT_DIM, BS], F32, tag="ps")
    psA = ps[0:OUT_DIM, :]
    psB = ps[OUT_DIM : 2 * OUT_DIM, :]

    def mm(half, k, j, c, start, stop):
        lhsT = W16[:, k * CW : (k + 1) * CW]
        if variant == "noew":
            rhs = U8[:, c * BS : (c + 1) * BS]
        else:
            rhs = Bj[j][:, c * BS : (c + 1) * BS]
        nc.tensor.matmul(half, lhsT, rhs, start=start, stop=stop,
                         skip_group_check=True)

    a_cnt = 0
    b_cnt = 0
    n_a = 32
    n_b = 31
    for j in range(8) if do_mm else ():
        for ca, cb in ((0, 7), (1, 6), (2, 5), (3, 4)):
            ka = 8 * ca + j
            mm(psA, ka, j, ca, start=(a_cnt == 0), stop=(a_cnt == n_a - 1))
            a_cnt += 1
            kb = 8 * cb + j
            if _valid(cb, j):
                mm(psB, kb, j, cb, start=(b_cnt == 0), stop=(b_cnt == n_b - 1))
                b_cnt += 1

    # ---- merge halves + bias, store
    yt = ypool.tile([OUT_DIM, BS], F32, tag="yt")
    if do_mm:
        nc.scalar.activation(yt[:], psB, ACTF.Identity, bias=bt[:, :])
        nc.vector.tensor_tensor(out=yt[:], in0=yt[:], in1=psA, op=ALU.add)
    else:
        nc.vector.memset(yt[:], 0.0)
    nc.sync.dma_start(out=yT[:, :], in_=yt[:])


_NC_CACHE = {}


def _get_program():
    if "nc" not in _NC_CACHE:
        _NC_CACHE["nc"] = build_program()
    return _NC_CACHE["nc"]


def make_in_maps(x, coeff, bias):
    x = np.ascontiguousarray(np.asarray(x, dtype=np.float32))
    coeff_r = np.ascontiguousarray(
        np.asarray(coeff, dtype=np.float32).reshape(IN_DIM, GRID * CW)
    )
    bias_r = np.ascontiguousarray(
        np.asarray(bias, dtype=np.float32).reshape(OUT_DIM, 1)
    )
    in_maps = []
    for c in range(N_CORES):
        xs = np.ascontiguousarray(x[c * BS : (c + 1) * BS, :].T)
        in_maps.append({"xT": xs, "coeff": coeff_r, "bias": bias_r})
    return in_maps


def kernel(x, coeff, bias):
    nc = _get_program()
    in_maps = make_in_maps(x, coeff, bias)
    res = run_bass_kernel_spmd(nc, in_maps, list(range(N_CORES)))
    y = np.concatenate([r["yT"].T for r in res.results], axis=0)
    return np.ascontiguousarray(y.astype(np.float32))


if __name__ == "__main__":
    xx = np.random.randn(B, IN_DIM).astype(np.float32)
    cc = (np.random.randn(IN_DIM, GRID, OUT_DIM) * 0.02).astype(np.float32)
    bb = np.zeros(OUT_DIM, dtype=np.float32)
    yy = kernel(xx, cc, bb)
    print("kernel output:", yy.shape, yy.dtype, float(np.abs(yy).mean()))
